# revision 1
# baseline (speedup 1.0000x reference)
"""Trainium2 Bass kernel for nn_DotAttention_19765439497049.

reference math:
    nq  = l2norm(query, -1)              # [B, Q, D]
    nk  = l2norm(key, -1)                # [B, W, S, D]
    sim = einsum('bqd,bwsd->bqws', nq, nk)
    sim = l2norm(sim, -1)                # normalize over S
    out = einsum('bqws,bwsd->bqwd', sim, key)

Key identity: the query normalization cancels inside the S-axis l2norm
(uniform positive scale per (b, q)), so the kernel never normalizes the
query.

Sharding: 8 cores = 4 batches x 2 query-halves. Each core handles
query[b, h*2048:(h+1)*2048] with the full (tiny) key[b]; outputs are
disjoint so the gather is a pure reshape.

Matmuls run in float32r (hw-rounded fp32, full PE rate at moving-dim>=256,
~1.4e-4 rel err on K=512 dots); the normalization path stays fp32.
"""

import numpy as np

B, Q, W, S, D = 4, 4096, 5, 64, 512
NCORES = 8
QSH = Q * B // NCORES      # 2048 queries per core
PT = 128                   # queries per inner tile
NT = QSH // PT             # 16 inner tiles
WS = W * S                 # 320
KC = D // 128              # 4 contraction chunks
LD_B = 4                   # q-tiles per input DMA  (1 MiB)
ST_B = 2                   # q-tiles per output DMA (2.5 MiB)

_CACHE = {}


def _build():
    import concourse.bacc as bacc
    import concourse.tile as tile
    from concourse import mybir, masks

    f32 = mybir.dt.float32
    f32r = mybir.dt.float32r
    AF = mybir.ActivationFunctionType

    nc = bacc.Bacc("TRN2", target_bir_lowering=False, debug=False)
    # chunk-major host-transposed query: qT[c, dl, q] = query[q, c*128+dl]
    qT_d = nc.dram_tensor("qT", [KC, 128, QSH], f32, kind="ExternalInput").ap()
    # chunk-major host-normalized+transposed key for matmul-1
    nkT_d = nc.dram_tensor("nkT", [KC, 128, WS], f32, kind="ExternalInput").ap()
    k_d = nc.dram_tensor("key", [WS, D], f32, kind="ExternalInput").ap()
    o_d = nc.dram_tensor("out", [QSH, W * D], f32, kind="ExternalOutput").ap()

    with tile.TileContext(nc) as tc:
        with (
            tc.tile_pool(name="const", bufs=1) as constp,
            tc.tile_pool(name="keyp", bufs=1) as keyp,
            tc.tile_pool(name="qin", bufs=4) as qin,
            tc.tile_pool(name="work", bufs=5) as work,
            tc.tile_pool(name="outp", bufs=3) as outp,
            tc.tile_pool(name="ps_sim", bufs=2, space="PSUM") as ps_sim,
            tc.tile_pool(name="ps_st", bufs=1, space="PSUM") as ps_st,
            tc.tile_pool(name="ps_o", bufs=4, space="PSUM") as ps_o,
        ):
            ident = constp.tile([128, 128], f32)
            masks.make_identity(nc, ident[:])
            ident_r = constp.tile([128, 128], f32r)
            nc.vector.tensor_copy(ident_r[:], ident[:])

            # warm the PE while the first DMAs are in flight
            warm_ps = ps_sim.tile([PT, WS], f32, tag="sim_ps")
            for i in range(8):
                nc.tensor.transpose(warm_ps[:, (i % 2) * 128:(i % 2 + 1) * 128],
                                    ident[:], ident[:])
            # force the ACT function-table loads off the critical path:
            # touch every activation function the kernel uses, tiny shapes
            dummy = constp.tile([1, 8], f32)
            nc.scalar.square(dummy[:, 0:2], ident[0:1, 0:2])
            nc.scalar.sqrt(dummy[:, 2:4], ident[0:1, 0:2])
            nc.scalar.mul(dummy[:, 4:6], ident[0:1, 0:2], 1.0)

            # ---- key loads (small; gate matmul-1/2) ---------------------
            nkT_sb = keyp.tile([128, KC * WS], f32, tag="nkT_sb")
            nkT = keyp.tile([128, KC * WS], f32r, tag="nkT")
            for h in range(2):
                cs, ce = h * 2, h * 2 + 2
                nc.sync.dma_start(
                    nkT_sb[:, cs * WS:ce * WS]
                    .rearrange("p (c n) -> p c n", n=WS),
                    nkT_d[cs:ce, :, :].rearrange("c p n -> p c n"))
                nc.vector.tensor_copy(nkT[:, cs * WS:ce * WS],
                                      nkT_sb[:, cs * WS:ce * WS])

            rows = [128, 128, 64]
            offs = [0, 128, 256]
            kr = []
            for r, (p, o) in enumerate(zip(rows, offs)):
                krt = keyp.tile([p, D], f32, tag=f"kr{r}")
                nc.sync.dma_start(krt[:], k_d[o:o + p, :])
                kr.append(krt)
            # raw key per w as f32r for matmul-2 moving operand: [64, 512]
            key_w = []
            for w in range(W):
                kw = keyp.tile([S, D], f32r, tag=f"keyw{w}")
                nc.vector.tensor_copy(kw[:], kr[w // 2][(w % 2) * S:(w % 2) * S + S, :])
                key_w.append(kw)

            # ---- main loop over query tiles -----------------------------
            # first groups are small (latency: get the pipeline started),
            # later ones big (DMA efficiency)
            groups = [[0], [1], [2], [3]] + \
                [[a, a + 1] for a in range(4, NT - 2, 2)] + [[NT - 2], [NT - 1]]
            g_of_t = {}
            for g in groups:
                for i, t in enumerate(g):
                    g_of_t[t] = (g, i)
            ld_groups = [[0], [1], [2, 3]] + \
                [list(range(a, a + 4)) for a in range(4, NT, 4)]
            ldg_of_t = {}
            for g in ld_groups:
                for i, t in enumerate(g):
                    ldg_of_t[t] = (g, i)
            qg = qr = None
            ld_span = 0
            ob = None
            for t in range(NT):
                ldg, ldi = ldg_of_t[t]
                if ldi == 0:
                    n = len(ldg)
                    ld_span = n * PT
                    qg = qin.tile([128, KC * LD_B * PT], f32, tag="qg")
                    nc.sync.dma_start(
                        qg[:, 0:KC * ld_span]
                        .rearrange("p (c q) -> p c q", q=ld_span),
                        qT_d[:, :, t * PT:(t + n) * PT]
                        .rearrange("c p q -> p c q"))
                    # round the whole group to f32r once (SBUF->SBUF, 2x mode)
                    qr = qin.tile([128, KC * LD_B * PT], f32r, tag="qr")
                    nc.vector.tensor_copy(qr[:, 0:KC * ld_span],
                                          qg[:, 0:KC * ld_span])
                qoff = ldi * PT

                # mm1: sim[q, ws] = sum_D qT^T nkT
                sim_ps = ps_sim.tile([PT, WS], f32, tag="sim_ps")
                for c in range(KC):
                    nc.tensor.matmul(
                        sim_ps[:],
                        qr[:, c * ld_span + qoff:c * ld_span + qoff + PT],
                        nkT[:, c * WS:(c + 1) * WS],
                        start=(c == 0), stop=(c == KC - 1))

                # sim -> SBUF (f32r) for the transposes; the sim-norm factors
                # are computed in parallel and folded into the output copies
                # (sim l2norm commutes with matmul-2).
                simc = work.tile([PT, WS], f32r, tag="simc")
                nc.vector.tensor_copy(simc[:], sim_ps[:])

                sq = work.tile([PT, WS], f32, tag="sq")
                nc.scalar.square(sq[:], sim_ps[:])
                ssq = work.tile([PT, 8], f32, tag="ssq")
                nc.vector.reduce_sum(
                    out=ssq[:, 0:W],
                    in_=sq[:].rearrange("p (w s) -> p w s", s=S),
                    axis=mybir.AxisListType.X)
                nrm = work.tile([PT, 8], f32, tag="nrm")
                nc.scalar.sqrt(nrm[:, 0:W], ssq[:, 0:W])
                rinv = work.tile([PT, 8], f32, tag="rinv")
                nc.vector.reciprocal(rinv[:, 0:W], nrm[:, 0:W])

                # transpose each (unnormalized) w-slice -> [S, q] f32r
                st_ps = ps_st.tile([S, W * PT], f32r, tag="st_ps")
                for w in range(W):
                    nc.tensor.transpose(st_ps[:, w * PT:(w + 1) * PT],
                                        simc[:, w * S:(w + 1) * S],
                                        ident_r[:])
                simT = work.tile([S, W * PT], f32r, tag="simT")
                nc.scalar.copy(simT[:], st_ps[:])

                # mm2 per w; scaled copies alternate between DVE and ACT
                grp, gi = g_of_t[t]
                if gi == 0:
                    ob = outp.tile([PT, ST_B * W * D], f32, tag="ob")
                obase = gi * W * D
                for w in range(W):
                    o_ps = ps_o.tile([PT, D], f32, tag="o_ps")
                    nc.tensor.matmul(o_ps[:], simT[:, w * PT:(w + 1) * PT],
                                     key_w[w][:], start=True, stop=True)
                    dst = ob[:, obase + w * D:obase + (w + 1) * D]
                    if (t * W + w) % 2 == 0:
                        nc.vector.tensor_scalar_mul(dst, o_ps[:], rinv[:, w:w + 1])
                    else:
                        nc.scalar.mul(dst, o_ps[:], rinv[:, w:w + 1])
                    if t < 4:
                        # pipeline-fill tiles: stream each w out immediately
                        nc.sync.dma_start(
                            o_d[t * PT:(t + 1) * PT, w * D:(w + 1) * D], dst)

                if t >= 4 and gi == len(grp) - 1:
                    n = len(grp)
                    dst = o_d[grp[0] * PT:(grp[0] + n) * PT, :] \
                        .rearrange("(j p) d -> p j d", p=PT)
                    nc.sync.dma_start(
                        dst,
                        ob[:, 0:n * W * D].rearrange("p (j d) -> p j d", d=W * D))

    nc.compile()
    return nc


def _get_nc():
    if "nc" not in _CACHE:
        _CACHE["nc"] = _build()
    return _CACHE["nc"]


def kernel(query: np.ndarray, key: np.ndarray) -> np.ndarray:
    from concourse.bass_utils import run_bass_kernel_spmd

    query = np.asarray(query, dtype=np.float32)
    key = np.asarray(key, dtype=np.float32)
    assert query.shape == (B, Q, D) and key.shape == (B, W, S, D)

    nc = _get_nc()
    half = Q // 2
    in_maps = []
    for core in range(NCORES):
        b, h = divmod(core, 2)
        qs = query[b, h * half:(h + 1) * half, :]      # [QSH, D]
        kb = key[b].reshape(WS, D)
        n = np.linalg.norm(kb.astype(np.float64), axis=-1, keepdims=True)
        nk = (kb.astype(np.float64) / np.maximum(n, 1e-12)).astype(np.float32)
        in_maps.append({
            "qT": np.ascontiguousarray(qs.T).reshape(KC, 128, QSH),
            "nkT": np.ascontiguousarray(nk.T).reshape(KC, 128, WS),
            "key": np.ascontiguousarray(kb),
        })
    res = run_bass_kernel_spmd(nc, in_maps, list(range(NCORES)))
    out = np.empty((B, Q, W, D), dtype=np.float32)
    for core in range(NCORES):
        b, h = divmod(core, 2)
        out[b, h * half:(h + 1) * half] = \
            res.results[core]["out"].reshape(half, W, D)
    return out



# revision 6
# speedup vs baseline: 1.3787x; 1.3787x over previous
"""Trainium2 Bass kernel for nn_DotAttention_19765439497049.

reference math:
    nq  = l2norm(query, -1)              # [B, Q, D]
    nk  = l2norm(key, -1)                # [B, W, S, D]
    sim = einsum('bqd,bwsd->bqws', nq, nk)
    sim = l2norm(sim, -1)                # normalize over S
    out = einsum('bqws,bwsd->bqwd', sim, key)

Key identities:
  * the query normalization cancels inside the S-axis l2norm (uniform
    positive scale per (b, q)), so the kernel never normalizes the query;
  * the S-axis l2norm commutes with matmul-2, so outputs are scaled by
    1/||sim_w|| after the matmul.

All HBM traffic is bf16 (inputs rounded on host, output upcast on host),
which halves the wire bytes; internal accumulation stays fp32 in PSUM.

Structure per 128-query tile (sim is computed directly transposed):
  mm1T:  simT[s, q] = sum_d nkT^T q  (3 ws-blocks x 4 k-chunks, PSUM)
  simT'  PSUM->SBUF bf16 copy (gpsimd)
  sq'  = simT'*simT'              (DVE, bf16 2x/4x mode)
  ssq  = sq'^T @ halfmask         (PE: stationary=sq', moving=[128,2] mask
                                   -> ssq lands directly in [q, w] layout)
  nrm  = sqrt(ssq)  (ACT)   rinv = 1/nrm  (DVE)
  mm2:   out_w = simT'_w^T @ key_w  (PSUM), then scaled copies to bf16
         SBUF split across DVE/ACT/Pool, batched DMA to DRAM.

Sharding: 8 cores = 4 batches x 2 query-halves; outputs are disjoint so
the gather is a pure reshape.
"""

import numpy as np

B, Q, W, S, D = 4, 4096, 5, 64, 512
NCORES = 8
QSH = Q * B // NCORES      # 2048 queries per core
PT = 128                   # queries per inner tile
NT = QSH // PT             # 16 inner tiles
WS = W * S                 # 320
WSP = 384                  # ws padded to 3 full 128-blocks
KC = D // 128              # 4 contraction chunks
LD_B = 4                   # q-tiles per input DMA
ST_B = 2                   # q-tiles per output DMA

_CACHE = {}


def _build():
    import concourse.bacc as bacc
    import concourse.tile as tile
    from concourse import mybir, masks

    f32 = mybir.dt.float32
    bf16 = mybir.dt.bfloat16

    nc = bacc.Bacc("TRN2", target_bir_lowering=False, debug=False)
    # chunk-major host-transposed query: qT[c, dl, q] = query[q, c*128+dl]
    qT_d = nc.dram_tensor("qT", [KC, 128, QSH], bf16, kind="ExternalInput").ap()
    # chunk-major host-normalized+transposed key, ws padded 320->384 with 0
    nkT_d = nc.dram_tensor("nkT", [KC, 128, WSP], bf16, kind="ExternalInput").ap()
    k_d = nc.dram_tensor("key", [WS, D], bf16, kind="ExternalInput").ap()
    # halfmask[p, j] = 1 if (p < 64) == (j == 0) else 0
    hm_d = nc.dram_tensor("hmask", [128, 2], bf16, kind="ExternalInput").ap()
    o_d = nc.dram_tensor("out", [QSH, W * D], bf16, kind="ExternalOutput").ap()

    with tile.TileContext(nc) as tc:
        with (
            tc.tile_pool(name="const", bufs=1) as constp,
            tc.tile_pool(name="keyp", bufs=1) as keyp,
            tc.tile_pool(name="qin", bufs=4) as qin,
            tc.tile_pool(name="work", bufs=4) as work,
            tc.tile_pool(name="outp", bufs=3) as outp,
            tc.tile_pool(name="ps_sim", bufs=2, space="PSUM") as ps_sim,
            tc.tile_pool(name="ps_nrm", bufs=2, space="PSUM") as ps_nrm,
            tc.tile_pool(name="ps_o", bufs=4, space="PSUM") as ps_o,
        ):
            ident = constp.tile([128, 128], f32)
            masks.make_identity(nc, ident[:])

            # warm the PE while the first DMAs are in flight
            warm_ps = ps_o.tile([PT, D], f32, tag="o_ps")
            for i in range(8):
                nc.tensor.transpose(warm_ps[:, (i % 4) * 128:(i % 4 + 1) * 128],
                                    ident[:], ident[:])
            # touch ACT function tables off the critical path
            dummy = constp.tile([1, 8], f32)
            nc.scalar.sqrt(dummy[:, 0:2], ident[0:1, 0:2])
            nc.scalar.mul(dummy[:, 2:4], ident[0:1, 0:2], 1.0)

            # ---- small loads (gate matmul-1/2) --------------------------
            nkT = keyp.tile([128, KC * WSP], bf16, tag="nkT")
            nc.sync.dma_start(
                nkT[:].rearrange("p (c n) -> p c n", n=WSP),
                nkT_d[:, :, :].rearrange("c p n -> p c n"))
            hmask = keyp.tile([128, 2], bf16, tag="hmask")
            nc.sync.dma_start(hmask[:], hm_d[:])
            # key rows stacked in w-pairs: kpair[c][hs*64:(hs+1)*64] = key[w=2c+hs]
            kpair = []
            for c in range(3):
                rows = 128 if c < 2 else 64
                kw = keyp.tile([128, D], bf16, tag=f"keyp{c}")
                nc.sync.dma_start(kw[0:rows, :], k_d[c * 128:c * 128 + rows, :])
                kpair.append(kw)

            # ---- main loop over query tiles -----------------------------
            groups = [[0], [1], [2], [3]] + \
                [[a, a + 1] for a in range(4, NT - 2, 2)] + [[NT - 2], [NT - 1]]
            g_of_t = {}
            for g in groups:
                for i, t in enumerate(g):
                    g_of_t[t] = (g, i)
            ld_groups = [[0], [1], [2, 3]] + \
                [list(range(a, a + 4)) for a in range(4, NT, 4)]
            ldg_of_t = {}
            for g in ld_groups:
                for i, t in enumerate(g):
                    ldg_of_t[t] = (g, i)
            qg = None
            ld_span = 0
            ob = None
            for t in range(NT):
                ldg, ldi = ldg_of_t[t]
                if ldi == 0:
                    n = len(ldg)
                    ld_span = n * PT
                    qg = qin.tile([128, KC * LD_B * PT], bf16, tag="qg")
                    nc.sync.dma_start(
                        qg[:, 0:KC * ld_span]
                        .rearrange("p (c q) -> p c q", q=ld_span),
                        qT_d[:, :, t * PT:(t + n) * PT]
                        .rearrange("c p q -> p c q"))
                qoff = ldi * PT

                # mm1T: simT[s(ws-block), q] = sum_d nkT^T q, 3 blocks
                st_ps = ps_sim.tile([128, WSP], f32, tag="st_ps")
                for b in range(3):
                    for c in range(KC):
                        nc.tensor.matmul(
                            st_ps[:, b * 128:(b + 1) * 128],
                            nkT[:, c * WSP + b * 128:c * WSP + (b + 1) * 128],
                            qg[:, c * ld_span + qoff:c * ld_span + qoff + PT],
                            start=(c == 0), stop=(c == KC - 1))

                # simT' -> SBUF bf16 (ACT; gpsimd cannot read PSUM);
                # square on gpsimd (SBUF->SBUF is legal there)
                simT = work.tile([128, WSP], bf16, tag="simT")
                nc.scalar.copy(simT[:], st_ps[:])
                sq = work.tile([128, WSP], bf16, tag="sq")
                nc.gpsimd.tensor_mul(sq[:], simT[:], simT[:])

                # ssq in [q, w] layout via PE: stationary=sq chunk, moving=mask
                nrm2_ps = ps_nrm.tile([PT, 8], f32, tag="nrm2")
                for c in range(3):
                    nc.tensor.matmul(
                        nrm2_ps[:, 2 * c:2 * c + 2],
                        sq[:, c * 128:(c + 1) * 128],
                        hmask[:, 0:2], start=True, stop=True)
                nrm = work.tile([PT, 8], f32, tag="nrm")
                nc.scalar.sqrt(nrm[:, 0:6], nrm2_ps[:, 0:6])
                rinv = work.tile([PT, 8], f32, tag="rinv")
                nc.vector.reciprocal(rinv[:, 0:W], nrm[:, 0:W])

                # mm2 per w; scaled copies split DVE/ACT/Pool
                grp, gi = g_of_t[t]
                if gi == 0:
                    ob = outp.tile([PT, ST_B * W * D], bf16, tag="ob")
                obase = gi * W * D
                for w in range(W):
                    c, hs = divmod(w, 2)
                    o_ps = ps_o.tile([PT, D], f32, tag="o_ps")
                    nc.tensor.matmul(
                        o_ps[:],
                        simT[hs * S:(hs + 1) * S, c * 128:(c + 1) * 128],
                        kpair[c][hs * S:(hs + 1) * S, :], start=True, stop=True)
                    dst = ob[:, obase + w * D:obase + (w + 1) * D]
                    if w < 3:
                        nc.vector.tensor_scalar_mul(dst, o_ps[:], rinv[:, w:w + 1])
                    else:
                        nc.scalar.mul(dst, o_ps[:], rinv[:, w:w + 1])

                if gi == len(grp) - 1:
                    n = len(grp)
                    dst = o_d[grp[0] * PT:(grp[0] + n) * PT, :] \
                        .rearrange("(j p) d -> p j d", p=PT)
                    nc.sync.dma_start(
                        dst,
                        ob[:, 0:n * W * D].rearrange("p (j d) -> p j d", d=W * D))

    nc.compile()
    return nc


def _get_nc():
    if "nc" not in _CACHE:
        _CACHE["nc"] = _build()
    return _CACHE["nc"]


def kernel(query: np.ndarray, key: np.ndarray) -> np.ndarray:
    import ml_dtypes
    from concourse.bass_utils import run_bass_kernel_spmd

    bf16 = ml_dtypes.bfloat16
    query = np.asarray(query, dtype=np.float32)
    key = np.asarray(key, dtype=np.float32)
    assert query.shape == (B, Q, D) and key.shape == (B, W, S, D)

    nc = _get_nc()
    half = Q // 2
    hmask = np.zeros((128, 2), dtype=bf16)
    hmask[:64, 0] = 1
    hmask[64:, 1] = 1
    in_maps = []
    for core in range(NCORES):
        b, h = divmod(core, 2)
        qs = query[b, h * half:(h + 1) * half, :]      # [QSH, D]
        kb = key[b].reshape(WS, D)
        n = np.linalg.norm(kb.astype(np.float64), axis=-1, keepdims=True)
        nk = kb.astype(np.float64) / np.maximum(n, 1e-12)
        nkT = np.zeros((D, WSP), dtype=bf16)
        nkT[:, :WS] = nk.T.astype(bf16)
        in_maps.append({
            "qT": np.ascontiguousarray(qs.T).reshape(KC, 128, QSH).astype(bf16),
            "nkT": nkT.reshape(KC, 128, WSP),
            "key": np.ascontiguousarray(kb).astype(bf16),
            "hmask": hmask,
        })
    res = run_bass_kernel_spmd(nc, in_maps, list(range(NCORES)))
    out = np.empty((B, Q, W, D), dtype=np.float32)
    for core in range(NCORES):
        b, h = divmod(core, 2)
        out[b, h * half:(h + 1) * half] = \
            res.results[core]["out"].astype(np.float32).reshape(half, W, D)
    return out


# revision 13
# speedup vs baseline: 1.4571x; 1.0568x over previous
"""Trainium2 Bass kernel for nn_DotAttention_19765439497049.

reference math:
    nq  = l2norm(query, -1)              # [B, Q, D]
    nk  = l2norm(key, -1)                # [B, W, S, D]
    sim = einsum('bqd,bwsd->bqws', nq, nk)
    sim = l2norm(sim, -1)                # normalize over S
    out = einsum('bqws,bwsd->bqwd', sim, key)

Key identities:
  * the query normalization cancels inside the S-axis l2norm (uniform
    positive scale per (b, q)), so the kernel never normalizes the query;
  * the S-axis l2norm commutes with matmul-2, so outputs are scaled by
    1/||sim_w|| after the matmul.

All HBM traffic is bf16 (inputs rounded on host, output upcast on host),
which halves the wire bytes; internal accumulation stays fp32 in PSUM.

Per 128-query tile (sim is computed directly transposed):
  A(t):  mm1T simT[s, q] = sum_d nkT^T q   (3 ws-blocks x 4 k-chunks,
         PSUM), PSUM->SBUF bf16 copy (ACT), sq' = simT'*simT' (gpsimd)
  B1(t): ssq = sq' chunk^T @ halfmask  (PE: lands directly in [q, w]
         layout), nrm = sqrt(ssq) (ACT), rinv = 1/nrm (DVE)
  B2(t): mm2 out_w = simT'_w^T @ key_w (PSUM), scaled bf16 copies split
         DVE/ACT, batched DMA to DRAM.

Emission order per iteration is B1(t-1), A(t), B2(t-2): every engine
instruction's inputs are at least ~a tile old when it is decoded, so the
4-deep engine wait queues never back up into the sequencers (the PE/DVE
sequencers are otherwise the cadence limiter, not engine throughput).
All input DMAs are issued up front so store semaphore waits never block
load issue; the last tile streams out per-w to shorten the tail.

Sharding: 8 cores = 4 batches x 2 query-halves; outputs are disjoint so
the gather is a pure reshape.
"""

import numpy as np

B, Q, W, S, D = 4, 4096, 5, 64, 512
NCORES = 8
QSH = Q * B // NCORES      # 2048 queries per core
PT = 128                   # queries per inner tile
NT = QSH // PT             # 16 inner tiles
WS = W * S                 # 320
WSP = 384                  # ws padded to 3 full 128-blocks
KC = D // 128              # 4 contraction chunks
LD_B = 4                   # q-tiles per input DMA
ST_B = 2                   # q-tiles per output DMA

_CACHE = {}


def _build():
    import concourse.bacc as bacc
    import concourse.tile as tile
    from concourse import mybir, masks

    f32 = mybir.dt.float32
    bf16 = mybir.dt.bfloat16

    nc = bacc.Bacc("TRN2", target_bir_lowering=False, debug=False)
    qT_d = nc.dram_tensor("qT", [KC, 128, QSH], bf16, kind="ExternalInput").ap()
    nkT_d = nc.dram_tensor("nkT", [KC, 128, WSP], bf16, kind="ExternalInput").ap()
    k_d = nc.dram_tensor("key", [WS, D], bf16, kind="ExternalInput").ap()
    hm_d = nc.dram_tensor("hmask", [128, 2], bf16, kind="ExternalInput").ap()
    o_d = nc.dram_tensor("out", [QSH, W * D], bf16, kind="ExternalOutput").ap()

    ld_groups = [[0], [1], [2, 3]] + \
        [list(range(a, a + 4)) for a in range(4, NT, 4)]
    st_groups = [[0], [1], [2], [3]] + \
        [[a, a + 1] for a in range(4, NT - 2, 2)] + [[NT - 2], [NT - 1]]
    g_of_t = {}
    for g in st_groups:
        for i, t in enumerate(g):
            g_of_t[t] = (g, i)

    with tile.TileContext(nc) as tc:
        with (
            tc.tile_pool(name="const", bufs=1) as constp,
            tc.tile_pool(name="keyp", bufs=1) as keyp,
            tc.tile_pool(name="qin", bufs=len(ld_groups)) as qin,
            tc.tile_pool(name="work", bufs=4) as work,
            tc.tile_pool(name="outp", bufs=3) as outp,
            tc.tile_pool(name="ps_sim", bufs=2, space="PSUM") as ps_sim,
            tc.tile_pool(name="ps_nrm", bufs=2, space="PSUM") as ps_nrm,
            tc.tile_pool(name="ps_o", bufs=4, space="PSUM") as ps_o,
        ):
            ident = constp.tile([128, 128], f32)
            masks.make_identity(nc, ident[:])

            # warm the PE while the first DMAs are in flight
            warm_ps = ps_o.tile([PT, D], f32, tag="o_ps")
            for i in range(8):
                nc.tensor.transpose(warm_ps[:, (i % 4) * 128:(i % 4 + 1) * 128],
                                    ident[:], ident[:])
            dummy = constp.tile([1, 8], f32)
            nc.scalar.sqrt(dummy[:, 0:2], ident[0:1, 0:2])
            nc.scalar.mul(dummy[:, 2:4], ident[0:1, 0:2], 1.0)

            # ---- all input DMAs issued up front -------------------------
            qg_of_t = {}
            nkT = keyp.tile([128, KC * WSP], bf16, tag="nkT")
            hmask = keyp.tile([128, 2], bf16, tag="hmask")
            kpair = []
            for gi, g in enumerate(ld_groups):
                n = len(g)
                t0 = g[0]
                qg = qin.tile([128, KC * LD_B * PT], bf16, tag=f"qg{gi}")
                nc.sync.dma_start(
                    qg[:, 0:KC * n * PT]
                    .rearrange("p (c q) -> p c q", q=n * PT),
                    qT_d[:, :, t0 * PT:(t0 + n) * PT]
                    .rearrange("c p q -> p c q"))
                for i, t in enumerate(g):
                    qg_of_t[t] = (qg, n * PT, i * PT)
                if gi == 0:
                    nc.sync.dma_start(
                        nkT[:].rearrange("p (c n) -> p c n", n=WSP),
                        nkT_d[:, :, :].rearrange("c p n -> p c n"))
                    nc.sync.dma_start(hmask[:], hm_d[:])
                if gi == 1:
                    # key rows in w-pairs: kpair[c][hs*64:...] = key[w=2c+hs]
                    for c in range(3):
                        rows = 128 if c < 2 else 64
                        kw = keyp.tile([128, D], bf16, tag=f"keyp{c}")
                        nc.sync.dma_start(kw[0:rows, :],
                                          k_d[c * 128:c * 128 + rows, :])
                        kpair.append(kw)

            # ---- software-pipelined main loop ---------------------------
            simT_t = {}
            sq_t = {}
            rinv_t = {}
            ob = None

            def stage_a(t):
                qg, ld_span, qoff = qg_of_t[t]
                st_ps = ps_sim.tile([128, WSP], f32, tag="st_ps")
                for b in range(3):
                    for c in range(KC):
                        nc.tensor.matmul(
                            st_ps[:, b * 128:(b + 1) * 128],
                            nkT[:, c * WSP + b * 128:c * WSP + (b + 1) * 128],
                            qg[:, c * ld_span + qoff:c * ld_span + qoff + PT],
                            start=(c == 0), stop=(c == KC - 1))
                simT = work.tile([128, WSP], bf16, tag="simT")
                nc.scalar.copy(simT[:], st_ps[:])
                sq = work.tile([128, WSP], bf16, tag="sq")
                nc.gpsimd.tensor_mul(sq[:], simT[:], simT[:])
                simT_t[t] = simT
                sq_t[t] = sq

            def stage_b1(u):
                sq = sq_t.pop(u)
                nrm2_ps = ps_nrm.tile([PT, 8], f32, tag="nrm2")
                for c in range(3):
                    nc.tensor.matmul(
                        nrm2_ps[:, 2 * c:2 * c + 2],
                        sq[:, c * 128:(c + 1) * 128],
                        hmask[:, 0:2], start=True, stop=True)
                nrm = work.tile([PT, 8], f32, tag="nrm")
                nc.scalar.sqrt(nrm[:, 0:6], nrm2_ps[:, 0:6])
                rinv = work.tile([PT, 8], f32, tag="rinv")
                nc.vector.reciprocal(rinv[:, 0:W], nrm[:, 0:W])
                rinv_t[u] = rinv

            def stage_b2(u):
                nonlocal ob
                simT = simT_t.pop(u)
                rinv = rinv_t.pop(u)
                grp, gi = g_of_t[u]
                if gi == 0:
                    ob = outp.tile([PT, ST_B * W * D], bf16, tag="ob")
                obase = gi * W * D
                last = (u == NT - 1)
                for w in range(W):
                    c, hs = divmod(w, 2)
                    o_ps = ps_o.tile([PT, D], f32, tag="o_ps")
                    nc.tensor.matmul(
                        o_ps[:],
                        simT[hs * S:(hs + 1) * S, c * 128:(c + 1) * 128],
                        kpair[c][hs * S:(hs + 1) * S, :], start=True, stop=True)
                    dst = ob[:, obase + w * D:obase + (w + 1) * D]
                    if w < 3:
                        nc.vector.tensor_scalar_mul(dst, o_ps[:], rinv[:, w:w + 1])
                    else:
                        nc.scalar.mul(dst, o_ps[:], rinv[:, w:w + 1])
                    if last:
                        # stream the last tile out per-w to shorten the tail
                        nc.sync.dma_start(
                            o_d[u * PT:(u + 1) * PT, w * D:(w + 1) * D], dst)
                if not last and gi == len(grp) - 1:
                    n = len(grp)
                    dstd = o_d[grp[0] * PT:(grp[0] + n) * PT, :] \
                        .rearrange("(j p) d -> p j d", p=PT)
                    nc.sync.dma_start(
                        dstd,
                        ob[:, 0:n * W * D].rearrange("p (j d) -> p j d", d=W * D))

            for t in range(NT):
                stage_a(t)
                if t >= 1:
                    stage_b1(t - 1)
                    stage_b2(t - 1)
            stage_b1(NT - 1)
            stage_b2(NT - 1)

    nc.compile()
    return nc


def _get_nc():
    if "nc" not in _CACHE:
        _CACHE["nc"] = _build()
    return _CACHE["nc"]


def kernel(query: np.ndarray, key: np.ndarray) -> np.ndarray:
    import ml_dtypes
    from concourse.bass_utils import run_bass_kernel_spmd

    bf16 = ml_dtypes.bfloat16
    query = np.asarray(query, dtype=np.float32)
    key = np.asarray(key, dtype=np.float32)
    assert query.shape == (B, Q, D) and key.shape == (B, W, S, D)

    nc = _get_nc()
    half = Q // 2
    hmask = np.zeros((128, 2), dtype=bf16)
    hmask[:64, 0] = 1
    hmask[64:, 1] = 1
    in_maps = []
    for core in range(NCORES):
        b, h = divmod(core, 2)
        qs = query[b, h * half:(h + 1) * half, :]      # [QSH, D]
        kb = key[b].reshape(WS, D)
        n = np.linalg.norm(kb.astype(np.float64), axis=-1, keepdims=True)
        nk = kb.astype(np.float64) / np.maximum(n, 1e-12)
        nkT = np.zeros((D, WSP), dtype=bf16)
        nkT[:, :WS] = nk.T.astype(bf16)
        in_maps.append({
            "qT": np.ascontiguousarray(qs.T).reshape(KC, 128, QSH).astype(bf16),
            "nkT": nkT.reshape(KC, 128, WSP),
            "key": np.ascontiguousarray(kb).astype(bf16),
            "hmask": hmask,
        })
    res = run_bass_kernel_spmd(nc, in_maps, list(range(NCORES)))
    out = np.empty((B, Q, W, D), dtype=np.float32)
    for core in range(NCORES):
        b, h = divmod(core, 2)
        out[b, h * half:(h + 1) * half] = \
            res.results[core]["out"].astype(np.float32).reshape(half, W, D)
    return out


# revision 15
# speedup vs baseline: 1.5385x; 1.0559x over previous
"""Trainium2 Bass kernel for nn_DotAttention_19765439497049.

reference math:
    nq  = l2norm(query, -1)              # [B, Q, D]
    nk  = l2norm(key, -1)                # [B, W, S, D]
    sim = einsum('bqd,bwsd->bqws', nq, nk)
    sim = l2norm(sim, -1)                # normalize over S
    out = einsum('bqws,bwsd->bqwd', sim, key)

Key identities:
  * the query normalization cancels inside the S-axis l2norm (uniform
    positive scale per (b, q)), so the kernel never normalizes the query;
  * the S-axis l2norm commutes with matmul-2, so outputs are scaled by
    1/||sim_w|| after the matmul.

All HBM traffic is bf16 (inputs rounded on host, output upcast on host),
which halves the wire bytes; internal accumulation stays fp32 in PSUM.

Per 128-query tile (sim is computed directly transposed):
  A(t):  mm1T simT[s, q] = sum_d nkT^T q   (3 ws-blocks x 4 k-chunks,
         PSUM), PSUM->SBUF bf16 copy (ACT), sq' = simT'*simT' (gpsimd)
  B1(t): ssq = sq' chunk^T @ halfmask  (PE: lands directly in [q, w]
         layout), nrm = sqrt(ssq) (ACT), rinv = 1/nrm (DVE)
  B2(t): mm2 out_w = simT'_w^T @ key_w (PSUM), scaled bf16 copies split
         DVE/ACT, batched DMA to DRAM.

Emission order per iteration is B1(t-1), A(t), B2(t-2): every engine
instruction's inputs are at least ~a tile old when it is decoded, so the
4-deep engine wait queues never back up into the sequencers (the PE/DVE
sequencers are otherwise the cadence limiter, not engine throughput).
All input DMAs are issued up front so store semaphore waits never block
load issue; the last tile streams out per-w to shorten the tail.

Sharding: 8 cores = 4 batches x 2 query-halves; outputs are disjoint so
the gather is a pure reshape.
"""

import numpy as np

B, Q, W, S, D = 4, 4096, 5, 64, 512
NCORES = 8
QSH = Q * B // NCORES      # 2048 queries per core
PT = 128                   # queries per inner tile
NT = QSH // PT             # 16 inner tiles
WS = W * S                 # 320
WSP = 384                  # ws padded to 3 full 128-blocks
KC = D // 128              # 4 contraction chunks
LD_B = 4                   # q-tiles per input DMA
ST_B = 2                   # q-tiles per output DMA

_CACHE = {}


def _build():
    import concourse.bacc as bacc
    import concourse.tile as tile
    from concourse import mybir, masks

    f32 = mybir.dt.float32
    bf16 = mybir.dt.bfloat16

    nc = bacc.Bacc("TRN2", target_bir_lowering=False, debug=False)
    qT_d = nc.dram_tensor("qT", [KC, 128, QSH], bf16, kind="ExternalInput").ap()
    nkT_d = nc.dram_tensor("nkT", [KC, 128, WSP], bf16, kind="ExternalInput").ap()
    k_d = nc.dram_tensor("key", [WS, D], bf16, kind="ExternalInput").ap()
    hm_d = nc.dram_tensor("hmask", [128, 2], bf16, kind="ExternalInput").ap()
    o_d = nc.dram_tensor("out", [QSH, W * D], bf16, kind="ExternalOutput").ap()

    ld_groups = [[0], [1], [2, 3]] + \
        [list(range(a, a + 4)) for a in range(4, NT, 4)]
    st_groups = [[0], [1], [2], [3]] + \
        [[a, a + 1] for a in range(4, NT - 2, 2)] + [[NT - 2], [NT - 1]]
    g_of_t = {}
    for g in st_groups:
        for i, t in enumerate(g):
            g_of_t[t] = (g, i)

    with tile.TileContext(nc) as tc:
        with (
            tc.tile_pool(name="const", bufs=1) as constp,
            tc.tile_pool(name="keyp", bufs=1) as keyp,
            tc.tile_pool(name="qin", bufs=len(ld_groups)) as qin,
            tc.tile_pool(name="work", bufs=4) as work,
            tc.tile_pool(name="outp", bufs=3) as outp,
            tc.tile_pool(name="ps_sim", bufs=2, space="PSUM") as ps_sim,
            tc.tile_pool(name="ps_nrm", bufs=2, space="PSUM") as ps_nrm,
            tc.tile_pool(name="ps_o", bufs=4, space="PSUM") as ps_o,
        ):
            ident = constp.tile([128, 128], f32)
            masks.make_identity(nc, ident[:])

            # warm the PE while the first DMAs are in flight
            warm_ps = ps_o.tile([PT, D], f32, tag="o_ps")
            for i in range(8):
                nc.tensor.transpose(warm_ps[:, (i % 4) * 128:(i % 4 + 1) * 128],
                                    ident[:], ident[:])
            dummy = constp.tile([1, 8], f32)
            nc.scalar.sqrt(dummy[:, 0:2], ident[0:1, 0:2])
            nc.scalar.mul(dummy[:, 2:4], ident[0:1, 0:2], 1.0)

            # ---- all input DMAs issued up front -------------------------
            qg_of_t = {}
            nkT = keyp.tile([128, KC * WSP], bf16, tag="nkT")
            hmask = keyp.tile([128, 2], bf16, tag="hmask")
            kpair = []
            for gi, g in enumerate(ld_groups):
                n = len(g)
                t0 = g[0]
                qg = qin.tile([128, KC * LD_B * PT], bf16, tag=f"qg{gi}")
                nc.sync.dma_start(
                    qg[:, 0:KC * n * PT]
                    .rearrange("p (c q) -> p c q", q=n * PT),
                    qT_d[:, :, t0 * PT:(t0 + n) * PT]
                    .rearrange("c p q -> p c q"))
                for i, t in enumerate(g):
                    qg_of_t[t] = (qg, n * PT, i * PT)
                if gi == 0:
                    nc.sync.dma_start(
                        nkT[:].rearrange("p (c n) -> p c n", n=WSP),
                        nkT_d[:, :, :].rearrange("c p n -> p c n"))
                    nc.sync.dma_start(hmask[:], hm_d[:])
                if gi == 1:
                    # key rows in w-pairs: kpair[c][hs*64:...] = key[w=2c+hs]
                    for c in range(3):
                        rows = 128 if c < 2 else 64
                        kw = keyp.tile([128, D], bf16, tag=f"keyp{c}")
                        nc.sync.dma_start(kw[0:rows, :],
                                          k_d[c * 128:c * 128 + rows, :])
                        kpair.append(kw)

            # ---- software-pipelined main loop ---------------------------
            simT_t = {}
            sq_t = {}
            rinv_t = {}
            ob = None

            st_t = {}

            def stage_a(t):
                qg, ld_span, qoff = qg_of_t[t]
                st_ps = ps_sim.tile([128, WSP], f32, tag="st_ps")
                for b in range(3):
                    for c in range(KC):
                        nc.tensor.matmul(
                            st_ps[:, b * 128:(b + 1) * 128],
                            nkT[:, c * WSP + b * 128:c * WSP + (b + 1) * 128],
                            qg[:, c * ld_span + qoff:c * ld_span + qoff + PT],
                            start=(c == 0), stop=(c == KC - 1))
                st_t[t] = st_ps

            def stage_a2(t):
                st_ps = st_t.pop(t)
                simT = work.tile([128, WSP], bf16, tag="simT")
                nc.scalar.copy(simT[:], st_ps[:])
                sq = work.tile([128, WSP], bf16, tag="sq")
                nc.gpsimd.tensor_mul(sq[:], simT[:], simT[:])
                simT_t[t] = simT
                sq_t[t] = sq

            def stage_b1(u):
                sq = sq_t.pop(u)
                nrm2_ps = ps_nrm.tile([PT, 8], f32, tag="nrm2")
                for c in range(3):
                    nc.tensor.matmul(
                        nrm2_ps[:, 2 * c:2 * c + 2],
                        sq[:, c * 128:(c + 1) * 128],
                        hmask[:, 0:2], start=True, stop=True)
                nrm = work.tile([PT, 8], f32, tag="nrm")
                nc.scalar.sqrt(nrm[:, 0:6], nrm2_ps[:, 0:6])
                rinv = work.tile([PT, 8], f32, tag="rinv")
                nc.vector.reciprocal(rinv[:, 0:W], nrm[:, 0:W])
                rinv_t[u] = rinv

            def stage_b2(u):
                nonlocal ob
                simT = simT_t.pop(u)
                rinv = rinv_t.pop(u)
                grp, gi = g_of_t[u]
                if gi == 0:
                    ob = outp.tile([PT, ST_B * W * D], bf16, tag="ob")
                obase = gi * W * D
                last = (u == NT - 1)
                for w in range(W):
                    c, hs = divmod(w, 2)
                    o_ps = ps_o.tile([PT, D], f32, tag="o_ps")
                    nc.tensor.matmul(
                        o_ps[:],
                        simT[hs * S:(hs + 1) * S, c * 128:(c + 1) * 128],
                        kpair[c][hs * S:(hs + 1) * S, :], start=True, stop=True)
                    dst = ob[:, obase + w * D:obase + (w + 1) * D]
                    if w < 3:
                        nc.vector.tensor_scalar_mul(dst, o_ps[:], rinv[:, w:w + 1])
                    else:
                        nc.scalar.mul(dst, o_ps[:], rinv[:, w:w + 1])
                    if last:
                        # stream the last tile out per-w to shorten the tail
                        nc.sync.dma_start(
                            o_d[u * PT:(u + 1) * PT, w * D:(w + 1) * D], dst)
                if not last and gi == len(grp) - 1:
                    n = len(grp)
                    dstd = o_d[grp[0] * PT:(grp[0] + n) * PT, :] \
                        .rearrange("(j p) d -> p j d", p=PT)
                    nc.sync.dma_start(
                        dstd,
                        ob[:, 0:n * W * D].rearrange("p (j d) -> p j d", d=W * D))

            for t in range(NT):
                stage_a(t)
                if t >= 1:
                    stage_a2(t - 1)
                if t >= 2:
                    stage_b1(t - 2)
                    stage_b2(t - 2)
            stage_a2(NT - 1)
            stage_b1(NT - 2)
            stage_b2(NT - 2)
            stage_b1(NT - 1)
            stage_b2(NT - 1)

    nc.compile()
    return nc


def _get_nc():
    if "nc" not in _CACHE:
        _CACHE["nc"] = _build()
    return _CACHE["nc"]


def kernel(query: np.ndarray, key: np.ndarray) -> np.ndarray:
    import ml_dtypes
    from concourse.bass_utils import run_bass_kernel_spmd

    bf16 = ml_dtypes.bfloat16
    query = np.asarray(query, dtype=np.float32)
    key = np.asarray(key, dtype=np.float32)
    assert query.shape == (B, Q, D) and key.shape == (B, W, S, D)

    nc = _get_nc()
    half = Q // 2
    hmask = np.zeros((128, 2), dtype=bf16)
    hmask[:64, 0] = 1
    hmask[64:, 1] = 1
    in_maps = []
    for core in range(NCORES):
        b, h = divmod(core, 2)
        qs = query[b, h * half:(h + 1) * half, :]      # [QSH, D]
        kb = key[b].reshape(WS, D)
        n = np.linalg.norm(kb.astype(np.float64), axis=-1, keepdims=True)
        nk = kb.astype(np.float64) / np.maximum(n, 1e-12)
        nkT = np.zeros((D, WSP), dtype=bf16)
        nkT[:, :WS] = nk.T.astype(bf16)
        in_maps.append({
            "qT": np.ascontiguousarray(qs.T).reshape(KC, 128, QSH).astype(bf16),
            "nkT": nkT.reshape(KC, 128, WSP),
            "key": np.ascontiguousarray(kb).astype(bf16),
            "hmask": hmask,
        })
    res = run_bass_kernel_spmd(nc, in_maps, list(range(NCORES)))
    out = np.empty((B, Q, W, D), dtype=np.float32)
    for core in range(NCORES):
        b, h = divmod(core, 2)
        out[b, h * half:(h + 1) * half] = \
            res.results[core]["out"].astype(np.float32).reshape(half, W, D)
    return out
